# revision 8
# baseline (speedup 1.0000x reference)
"""CrystalGraphAttention Trainium2 kernel (v4).

Data-parallel over batch: core b handles batch b (B=8, 8 cores).
Per-core algorithm (transposed layouts, [feature, node]):
  xT (host-pretransposed bf16)  ->  qT = Wq^T xT, kT = Wk^T xT (head pairs
  row-packed), v[tc] = (x Wv) chunks.
  Attention per (st half, head pair, t8 chunk); logits ls_e in single-bank
  PSUM tiles [128,512] per head half:
    qk pair row-tiled (K=64 tiles at (0,0)/(64,0)) -> 2x PE concurrency.
    exp path split between engines:
      ACT tiles: -1e9*mask injected via identity matmuls (PSUM accumulate),
        then ScalarE exp with per-partition dw scale.
      DVE tiles: no mask matmul; one scalar_tensor_tensor per half computes
        i16(ls * (A16*dw[t]) + (B16 - C16*mask)) -- Schraudolph exp emitted
        directly in the bf16 bit layout (A16=2^7/ln2), so the i16 result
        bitcast to bf16 IS exp(dw*qk) masked (masked entries ~2^-103), and
        the po/pd matmul rhs is contiguous.
    po pair col-tiled (0,0)/(0,64), pd (denominator) ones-matmul pair
    likewise; po/pd emission delayed one iteration so PE never waits on exp.
  normalize: reciprocal + 2 half-partition gpsimd broadcasts building a
  mixed [128,512] reciprocal tile + one tensor_tensor mult.
  output projection transposed (outT[d, n]); bias added on ScalarE during
  PSUM evacuation; host transposes the [D, N] result back.
"""
import os
import sys

if '/opt/trn_rl_repo' not in sys.path:
    sys.path.insert(0, '/opt/trn_rl_repo')

import numpy as np

B, N, D = 8, 1024, 256
H, DK, DV = 8, 64, 64
NCORES = 8

A16 = float(2.0 ** 7 / np.log(2.0))
B16 = float((127.0 - 0.044) * 2.0 ** 7)
MASKC = 1600.0  # masked-entry center: bf16 exponent <= 24 -> exp ~ 2^-103
C16 = B16 - MASKC

_COMPILED = {}

# DVE-path t8 sets per p_idx parity (rest go to ACT path)
DVE_T8_EVEN = (1, 2, 4, 6)
DVE_T8_ODD = (1, 2, 3, 5, 6)
PRE_T8 = tuple(sorted(set(DVE_T8_EVEN) | set(DVE_T8_ODD)))


def _build():
    import concourse.bass as bass
    import concourse.mybir as mybir
    import concourse.tile as tile
    from concourse import bacc
    from concourse.masks import make_identity

    f32 = mybir.dt.float32
    bf16 = mybir.dt.bfloat16
    i16 = mybir.dt.int16
    MULT = mybir.AluOpType.mult
    ADD = mybir.AluOpType.add
    EXP = mybir.ActivationFunctionType.Exp
    COPYF = mybir.ActivationFunctionType.Copy

    nc = bacc.Bacc(None, target_bir_lowering=False)

    xT_d = nc.dram_tensor("xT", [D, N], bf16, kind="ExternalInput")
    m_d = nc.dram_tensor("m", [N, N], bf16, kind="ExternalInput")
    dwc_d = nc.dram_tensor("dwcol", [128, 8], f32, kind="ExternalInput")
    adw_d = nc.dram_tensor("adwcol", [128, 8], f32, kind="ExternalInput")
    wq_d = nc.dram_tensor("wq", [D, H * DK], bf16, kind="ExternalInput")
    wk_d = nc.dram_tensor("wk", [D, H * DK], bf16, kind="ExternalInput")
    wv_d = nc.dram_tensor("wv", [D, H * DV], bf16, kind="ExternalInput")
    wo_d = nc.dram_tensor("wo", [128, 1024], bf16, kind="ExternalInput")
    bo_d = nc.dram_tensor("bo", [128, 2], f32, kind="ExternalInput")
    out_d = nc.dram_tensor("out", [D, N], f32, kind="ExternalOutput")

    with tile.TileContext(nc) as tc:
        with tc.tile_pool(name="const", bufs=1) as cst, \
             tc.tile_pool(name="big", bufs=1) as big, \
             tc.tile_pool(name="pre", bufs=2) as prepool, \
             tc.tile_pool(name="exq", bufs=8) as expool, \
             tc.tile_pool(name="sin", bufs=8) as sinpool, \
             tc.tile_pool(name="nrm", bufs=2) as npool, \
             tc.tile_pool(name="outp", bufs=3) as opool, \
             tc.tile_pool(name="psls", bufs=5, space="PSUM") as ps_ls, \
             tc.tile_pool(name="pspo", bufs=2, space="PSUM") as ps_po, \
             tc.tile_pool(name="pssm", bufs=1, space="PSUM") as ps_sm:

            # ---- weights / inputs (spread across the 3 DMA queues) ----
            wq_r = big.tile([128, 1024], bf16, name="wq_r")
            nc.scalar.dma_start(
                wq_r.rearrange("p (kd c) -> p kd c", kd=2),
                wq_d.rearrange("(kd p) c -> p kd c", p=128))
            xT = big.tile([128, 2048], bf16, name="xT")  # [p, kd*1024 + n]
            xTr = xT.rearrange("p (kd n) -> p kd n", kd=2)
            xdr = xT_d.rearrange("(kd p) n -> p kd n", p=128)
            nc.scalar.dma_start(xTr[:, :, 0:512], xdr[:, :, 0:512])
            nc.scalar.dma_start(xTr[:, :, 512:1024], xdr[:, :, 512:1024])
            wk_r = big.tile([128, 1024], bf16, name="wk_r")
            nc.gpsimd.dma_start(
                wk_r.rearrange("p (kd c) -> p kd c", kd=2),
                wk_d.rearrange("(kd p) c -> p kd c", p=128))
            wv_r = big.tile([128, 1024], bf16, name="wv_r")
            nc.gpsimd.dma_start(
                wv_r.rearrange("p (kd c) -> p kd c", kd=2),
                wv_d.rearrange("(kd p) c -> p kd c", p=128))
            wo_r = big.tile([128, 1024], bf16, name="wo_r")
            nc.gpsimd.dma_start(wo_r, wo_d[:, :])

            dwc = cst.tile([128, 8], f32)
            nc.gpsimd.dma_start(dwc, dwc_d[:, :])
            adw = cst.tile([128, 8], f32)
            nc.gpsimd.dma_start(adw, adw_d[:, :])
            bo_c = cst.tile([128, 2], f32)
            nc.gpsimd.dma_start(bo_c, bo_d[:, :])

            mTn = [big.tile([128, N], bf16, name=f"mTn{t8}") for t8 in range(8)]
            for t8 in range(8):
                nc.sync.dma_start(mTn[t8], m_d[t8 * 128:(t8 + 1) * 128, :])

            # ---- constants ----
            ident = cst.tile([128, 128], f32)
            make_identity(nc, ident)
            idn_f = cst.tile([128, 128], f32)
            nc.vector.tensor_scalar_mul(idn_f, ident, -1.0e9)
            idn = cst.tile([128, 128], bf16)
            nc.vector.tensor_copy(idn, idn_f)
            ones_c = cst.tile([128, 1], f32)
            nc.vector.memset(ones_c, 1.0)
            ones_bf = cst.tile([128, 1], bf16)
            nc.vector.tensor_copy(ones_bf, ones_c)

            # ---- pre-mask tiles (ScalarE): pre = B16 - C16*m ----
            pre2 = {}

            def build_pre(st, t8):
                t = prepool.tile([128, 512], f32, tag=f"pre_{t8}",
                                 name=f"pre{st}_{t8}")
                nc.scalar.activation(
                    t, mTn[t8][:, st * 512:st * 512 + 512], COPYF,
                    bias=B16, scale=-C16)
                pre2[(st, t8)] = t

            for t8 in PRE_T8:
                build_pre(0, t8)

            # ---- qT, kT projections ----
            qT = big.tile([128, 4 * N], bf16)  # [dk + 64*(h%2), (h//2)*N + n]
            kT = big.tile([128, 4 * N], bf16)
            for c4 in range(4):
                for nt in range(2):
                    psq = ps_ls.tile([128, 512], f32, tag="ls")
                    psk = ps_ls.tile([128, 512], f32, tag="ls")
                    for kd in range(2):
                        nc.tensor.matmul(
                            psq,
                            wq_r[:, kd * 512 + c4 * 128:kd * 512 + (c4 + 1) * 128],
                            xT[:, kd * N + nt * 512:kd * N + nt * 512 + 512],
                            start=(kd == 0), stop=(kd == 1))
                    for kd in range(2):
                        nc.tensor.matmul(
                            psk,
                            wk_r[:, kd * 512 + c4 * 128:kd * 512 + (c4 + 1) * 128],
                            xT[:, kd * N + nt * 512:kd * N + nt * 512 + 512],
                            start=(kd == 0), stop=(kd == 1))
                    off = c4 * N + nt * 512
                    nc.scalar.copy(qT[:, off:off + 512], psq)
                    nc.scalar.copy(kT[:, off:off + 512], psk)

            # ---- v chunks ----
            vaug = [big.tile([128, 512], bf16, name=f"v{t8}") for t8 in range(8)]
            for t8 in range(8):
                psv = ps_sm.tile([128, 512], f32, tag="sm")
                for kd in range(2):
                    nc.tensor.matmul(
                        psv, xT[:, kd * N + t8 * 128:kd * N + (t8 + 1) * 128],
                        wv_r[:, kd * 512:(kd + 1) * 512],
                        start=(kd == 0), stop=(kd == 1))
                nc.scalar.copy(vaug[t8], psv)

            # ---- attention ----
            oT = [big.tile([128, 4 * 512], bf16, name=f"oT{st}")
                  for st in range(2)]

            def emit_outproj(st, dh):
                psp = ps_sm.tile([128, 512], f32, tag="sm",
                                 name=f"psp_{st}_{dh}")
                for cc in range(4):
                    nc.tensor.matmul(
                        psp, wo_r[:, (cc * 2 + dh) * 128:(cc * 2 + dh + 1) * 128],
                        oT[st][:, cc * 512:(cc + 1) * 512],
                        start=(cc == 0), stop=(cc == 3))
                osb = opool.tile([128, 512], f32, tag="outp")
                nc.scalar.add(osb, psp, bo_c[:, dh:dh + 1])
                nc.gpsimd.dma_start(
                    out_d[dh * 128:(dh + 1) * 128, st * 512:st * 512 + 512], osb)

            pending = []  # (po, pd, exv, first, last)

            def flush_pending():
                while pending:
                    po, pd, exv, first, last = pending.pop(0)
                    for e in range(2):
                        h_off = e * 64
                        nc.tensor.matmul(
                            po[h_off:h_off + 64, :], exv[e + 2],
                            exv[e], start=first, stop=last,
                            tile_position=(0, h_off))
                    for e in range(2):
                        nc.tensor.matmul(
                            pd[64 * e:64 * e + 1, :], ones_bf,
                            exv[e], start=first, stop=last,
                            tile_position=(0, 64 * e))

            deferred = [None]

            def make_norm(st, p_idx, po, pd):
                def norm():
                    rden = npool.tile([128, 512], f32, tag="dsb")
                    nc.vector.reciprocal_approx_fast(rden, pd)
                    rr0 = npool.tile([1, 512], f32, tag="rr0")
                    rr1 = npool.tile([1, 512], f32, tag="rr1")
                    nc.sync.dma_start(rr0, rden[0:1, :])
                    nc.sync.dma_start(rr1, rden[64:65, :])
                    rb = npool.tile([128, 512], f32, tag="rb")
                    rbb = npool.tile([128, 512], f32, tag="rbb")
                    nc.gpsimd.partition_broadcast(rb, rr0)
                    nc.gpsimd.partition_broadcast(rbb, rr1)
                    nc.sync.dma_start(rb[64:128, :], rbb[64:128, :])
                    nc.vector.tensor_tensor(
                        oT[st][:, p_idx * 512:(p_idx + 1) * 512], po, rb, MULT)
                    # interleave previous st's output projection
                    if st == 1 and p_idx < 2:
                        emit_outproj(0, p_idx)
                return norm

            # st=1 pre-build schedule: one per iteration late in st=0
            pre1_sched = {}
            for i, t8v in enumerate(PRE_T8):
                pre1_sched[(2 + i // 4, (1 + 2 * i) % 8)] = t8v

            for st in range(2):
                for p_idx in range(4):
                    co = p_idx * N
                    dve_set = DVE_T8_EVEN if p_idx % 2 == 0 else DVE_T8_ODD
                    po = ps_po.tile([128, 512], f32, tag="po",
                                    name=f"po_{st}_{p_idx}")
                    pd = ps_sm.tile([128, 512], f32, tag="sm",
                                    name=f"pd_{st}_{p_idx}")
                    for t8 in range(8):
                        first, last = (t8 == 0), (t8 == 7)
                        use_dve = t8 in dve_set
                        lsh = [ps_ls.tile([128, 512], f32, tag="ls",
                                          name=f"ls{e}_{st}_{p_idx}_{t8}")
                               for e in range(2)]
                        if not use_dve:
                            for e in range(2):
                                nc.tensor.matmul(
                                    lsh[e], idn,
                                    mTn[t8][:, st * 512:st * 512 + 512],
                                    start=True, stop=False)
                        for e in range(2):
                            nc.tensor.matmul(
                                lsh[e],
                                kT[e * 64:(e + 1) * 64,
                                   co + t8 * 128:co + (t8 + 1) * 128],
                                qT[e * 64:(e + 1) * 64,
                                   co + st * 512:co + st * 512 + 512],
                                start=use_dve, stop=True)
                        # flush previous iteration's po/pd (behind this
                        # iteration's matmuls in the PE queue), then any
                        # deferred normalize
                        flush_pending()
                        if deferred[0] is not None:
                            deferred[0]()
                            deferred[0] = None
                        exv = []
                        if use_dve:
                            for e in range(2):
                                sint = sinpool.tile([128, 512], i16, tag="sin")
                                nc.vector.scalar_tensor_tensor(
                                    sint, lsh[e], adw[:, t8:t8 + 1],
                                    pre2[(st, t8)], MULT, ADD)
                                exv.append(sint.bitcast(bf16))
                        else:
                            for e in range(2):
                                ext = expool.tile([128, 512], bf16, tag="exq")
                                nc.scalar.activation(ext, lsh[e], EXP,
                                                     scale=dwc[:, t8:t8 + 1])
                                exv.append(ext)
                        exv.append(vaug[t8][:, (2 * p_idx) * 64:
                                            (2 * p_idx) * 64 + 64])
                        exv.append(vaug[t8][:, (2 * p_idx + 1) * 64:
                                            (2 * p_idx + 1) * 64 + 64])
                        pending.append((po, pd, exv, first, last))
                        if st == 0 and (p_idx, t8) in pre1_sched:
                            build_pre(1, pre1_sched[(p_idx, t8)])
                    deferred[0] = make_norm(st, p_idx, po, pd)
                # end p_idx loop
            flush_pending()
            deferred[0]()
            emit_outproj(1, 0)
            emit_outproj(1, 1)

    nc.compile()
    return nc


def _get_compiled():
    if 'nc' not in _COMPILED:
        _COMPILED['nc'] = _build()
    return _COMPILED['nc']


def _shard(inputs):
    import ml_dtypes
    bf = ml_dtypes.bfloat16
    x = np.ascontiguousarray(inputs['node_features'], dtype=np.float32)
    em = np.ascontiguousarray(inputs['edge_mask'], dtype=np.float32)
    dw = np.ascontiguousarray(inputs['distance_weights'], dtype=np.float32)
    wq = (np.float32(0.125) * inputs['Wq']).astype(bf)
    wk = np.ascontiguousarray(inputs['Wk'], dtype=np.float32).astype(bf)
    wv = np.ascontiguousarray(inputs['Wv'], dtype=np.float32).astype(bf)
    wo = np.ascontiguousarray(inputs['Wo'], dtype=np.float32)
    wo_r = np.ascontiguousarray(
        wo.reshape(4, 128, 2, 128).transpose(1, 0, 2, 3).reshape(128, 1024)
    ).astype(bf)
    bo = np.ascontiguousarray(inputs['bo'], dtype=np.float32)
    bo_c = np.ascontiguousarray(bo.reshape(2, 128).T)
    maps = []
    for b in range(NCORES):
        m_bf = np.ascontiguousarray(1.0 - em[b, 0].T).astype(bf)
        dwcol = np.ascontiguousarray(dw[b].reshape(8, 128).T)
        maps.append({
            "xT": np.ascontiguousarray(x[b].T).astype(bf),
            "m": m_bf,
            "dwcol": dwcol,
            "adwcol": np.ascontiguousarray(dwcol * np.float32(A16)),
            "wq": wq, "wk": wk, "wv": wv, "wo": wo_r, "bo": bo_c,
        })
    return maps


def run_sharded(inputs, **kwargs):
    from concourse.bass_utils import run_bass_kernel_spmd
    nc = _get_compiled()
    maps = _shard(inputs)
    res = run_bass_kernel_spmd(nc, maps, core_ids=list(range(NCORES)), **kwargs)
    out = np.stack([res.results[b]["out"].T for b in range(NCORES)], axis=0)
    return np.ascontiguousarray(out), res


def kernel(**inputs) -> np.ndarray:
    out, _ = run_sharded(inputs)
    return out
